# revision 11
# baseline (speedup 1.0000x reference)
"""Depthwise 4x4 separable blur (upfirdn2d pad=(2,1)) on 8 TRN2 NeuronCores.

v3 design — h-pair partitioning, split DMA queues:
  - Pure data parallel over batch: core b gets image b = [C=128, H=256, W=256].
  - SBUF partition p holds H-row pair (2p, 2p+1): every DMA descriptor is a
    2 KB contiguous run, and the whole H extent lives in one tile so the
    H-conv needs no cross-tile seam handling.
  - Input DMAs issue from the Sync HWDGE ring, output DMAs + weight loads
    from the Scalar HWDGE ring, so input prefetch is never queued behind
    output triggers that wait on compute.
  - W-pass on VectorE: two flat scalar_tensor_tensor ops over the whole
    tile (reads that cross a 256-column row boundary produce garbage that
    3 tiny strided DVE fixup ops overwrite — the fixup APs cover every
    boundary). Outputs t1/t2 in fp16.
  - H-pass on TensorE (fp16, 1 cyc/row, fast weight load): for parities
    e, e' the banded matrices B[q][e][ep][p, m] = scale_q * kh[(2p+e) -
    (2m+ep) + 2] give psum[ep] = sum_{q, e} B^T tq[e], PSUM-accumulated.
  - ScalarE interleaves PSUM -> SBUF in (c, e', w) order so output DMA
    descriptors are also 2 KB contiguous.
"""

import os
import sys

import numpy as np

for _p in ("/opt/trn_rl_repo", "/root/.axon_site/_ro/trn_rl_repo"):
    if os.path.isdir(_p) and _p not in sys.path:
        sys.path.append(_p)

import concourse.bacc as bacc
import concourse.mybir as mybir
from concourse import tile
from concourse.bass_utils import run_bass_kernel_spmd

B, C, H, W = 8, 128, 256, 256
N_CORES = 8
CG = 8               # channels per inner tile group
NG = C // CG         # groups
HP = H // 2          # 128 h-pairs = partitions
EW = 2 * W           # flat (e, w) extent per (partition, channel) = 512
FG = CG * EW         # free elements per x/t tile
KS = 4
MM_DT = mybir.dt.float16
OC = 4               # channels per output staging tile / DMA


def _build_bands(kern: np.ndarray):
    """Factor flip(kern) = outer(kh, kw); build the 8 parity band matrices."""
    k = np.flip(kern.astype(np.float64), (0, 1))
    u, s, vt = np.linalg.svd(k)
    assert s[1] < 1e-6 * s[0], "blur kernel must be separable"
    kh = u[:, 0] * np.sqrt(s[0])
    kw = vt[0] * np.sqrt(s[0])
    if kh.sum() < 0:
        kh, kw = -kh, -kw
    assert np.allclose(np.outer(kh, kw), k, atol=1e-12 + 1e-7 * np.abs(k).max())
    assert abs(kw[3]) > 1e-12 and abs(kw[2]) > 1e-12
    r1 = float(kw[0] / kw[3])   # t1 = r1 * x[w-2] + x[w+1]
    r2 = float(kw[1] / kw[2])   # t2 = r2 * x[w-1] + x[w]
    scales = (kw[3], kw[2])     # psum += scale_q * band^T tq

    M = np.zeros((H, H), np.float64)
    for hh in range(H):
        for t in range(KS):
            i = hh + t - 2
            if 0 <= i < H:
                M[i, hh] = kh[t]
    bands = np.zeros((2, 2, 2, HP, HP), np.float64)
    for q in range(2):
        for e in range(2):
            for ep in range(2):
                bands[q, e, ep] = scales[q] * M[e::2, ep::2]
    return bands.reshape(8, HP, HP).astype(np.float32), r1, r2


def _build_nc(r1: float, r2: float):
    nc = bacc.Bacc("TRN2", target_bir_lowering=False, debug=False,
                   num_devices=N_CORES)
    x = nc.dram_tensor("input", [C, H, W], mybir.dt.float32,
                       kind="ExternalInput").ap()
    bands = nc.dram_tensor("bands", [8, HP, HP], mybir.dt.float32,
                           kind="ExternalInput").ap()
    out = nc.dram_tensor("output", [C, H, W], mybir.dt.float32,
                         kind="ExternalOutput").ap()
    mult = mybir.AluOpType.mult
    add = mybir.AluOpType.add

    with tile.TileContext(nc) as tc:
        with (
            tc.tile_pool(name="bands", bufs=1) as bp,
            tc.tile_pool(name="xp", bufs=4) as xpp,
            tc.tile_pool(name="tp", bufs=3) as tp,
            tc.tile_pool(name="osb", bufs=4) as osb,
            tc.tile_pool(name="ps", bufs=8, space="PSUM") as pp,
        ):
            # Band matrices via the Scalar HWDGE ring; cast to fp16 on DVE.
            wm = {}
            for idx in range(8):
                bt = bp.tile([HP, HP], mybir.dt.float32, tag=f"bf{idx}")
                nc.scalar.dma_start(bt[:], bands[idx])
                br = bp.tile([HP, HP], MM_DT, tag=f"br{idx}")
                nc.vector.tensor_copy(br[:], bt[:])
                q, e, ep = idx >> 2, (idx >> 1) & 1, idx & 1
                wm[q, e, ep] = br

            # Taper first/last groups so pipeline fill and drain are short.
            segs = []
            c = 0
            for cg in [4, 4] + [CG] * ((C - 16) // CG) + [4, 4]:
                segs.append((c, cg))
                c += cg
            assert c == C
            for c0, cg in segs:
                fg = cg * EW
                xt = xpp.tile([HP, fg], mybir.dt.float32, tag="x")
                xf = xt[:]
                nc.sync.dma_start(
                    xf.rearrange("p (c f) -> p c f", c=cg),
                    x[c0:c0 + cg].rearrange("c (p e) w -> p c (e w)", e=2),
                )
                t1 = tp.tile([HP, fg], MM_DT, tag="t1")
                t2 = tp.tile([HP, fg], MM_DT, tag="t2")
                t1f, t2f = t1[:], t2[:]
                # Main W-pass: flat ranges over the whole tile; every
                # 256-boundary-corrupted column is rewritten by the fixups.
                nc.vector.scalar_tensor_tensor(
                    t1f[:, 2:fg - 1], xf[:, 0:fg - 3], r1,
                    xf[:, 3:fg], mult, add)
                nc.vector.scalar_tensor_tensor(
                    t2f[:, 1:fg], xf[:, 0:fg - 1], r2,
                    xf[:, 1:fg], mult, add)
                # Fixups (strided 4d views over c and both e rows):
                t1e = t1f.rearrange("p (c pr w) -> p c pr w", c=cg, pr=2)
                t2e = t2f.rearrange("p (c pr w) -> p c pr w", c=cg, pr=2)
                xe = xf.rearrange("p (c pr w) -> p c pr w", c=cg, pr=2)
                # t1[w=0,1] = x[w+1] (left pad kills the r1 term)
                nc.vector.tensor_copy(t1e[:, :, :, 0:2], xe[:, :, :, 1:3])
                # t1[w=255] = r1 * x[w-2] (right pad kills the + term)
                nc.vector.tensor_scalar_mul(
                    t1e[:, :, :, W - 1:W], xe[:, :, :, W - 3:W - 2], r1)
                # t2[w=0] = x[w] (left pad kills the r2 term)
                nc.vector.tensor_copy(t2e[:, :, :, 0:1], xe[:, :, :, 0:1])

                t1c = t1f.rearrange("p (c f) -> p c f", c=cg)
                t2c = t2f.rearrange("p (c f) -> p c f", c=cg)
                for s0 in range(0, cg, OC):
                    oc = min(OC, cg - s0)
                    ot = osb.tile([HP, oc * EW], mybir.dt.float32, tag="o")
                    oc4 = ot[:].rearrange("p (c e w) -> p c e w", c=oc, e=2)
                    for pr in range(oc // 2):
                        cc = s0 + pr * 2
                        for ep in (0, 1):
                            ps = pp.tile([HP, 512], mybir.dt.float32, tag="ps")
                            first = True
                            for q, tt in ((0, t1c), (1, t2c)):
                                for e in (0, 1):
                                    rhs = tt[:, cc:cc + 2,
                                             e * W:(e + 1) * W]
                                    nc.tensor.matmul(
                                        ps[:], wm[q, e, ep][:], rhs,
                                        start=first,
                                        stop=(q == 1 and e == 1))
                                    first = False
                            nc.scalar.copy(
                                oc4[:, pr * 2:pr * 2 + 2, ep, :],
                                ps[:].rearrange("p (c w) -> p c w", c=2),
                            )
                    nc.scalar.dma_start(
                        out[c0 + s0:c0 + s0 + oc]
                        .rearrange("c (p e) w -> p c (e w)", e=2),
                        oc4.rearrange("p c e w -> p c (e w)"),
                    )
    nc.compile()
    return nc


_CACHE = {}


def _get_nc(r1: float, r2: float):
    key = (r1, r2)
    if key not in _CACHE:
        _CACHE[key] = _build_nc(r1, r2)
    return _CACHE[key]


def kernel(**inputs) -> np.ndarray:
    x = np.asarray(inputs["input"], dtype=np.float32)
    kern = np.asarray(inputs["kernel"], dtype=np.float32)
    assert x.shape == (B, C, H, W) and kern.shape == (KS, KS)
    bands, r1, r2 = _build_bands(kern)
    nc = _get_nc(r1, r2)
    in_maps = [
        {"input": np.ascontiguousarray(x[i]), "bands": bands}
        for i in range(N_CORES)
    ]
    res = run_bass_kernel_spmd(nc, in_maps, list(range(N_CORES)))
    global _LAST_RESULTS
    _LAST_RESULTS = res
    return np.stack([res.results[i]["output"] for i in range(N_CORES)])


if __name__ == "__main__":
    rng = np.random.default_rng(0)
    x = rng.standard_normal((B, C, H, W), dtype=np.float32)
    k1 = np.array([1.0, 3.0, 3.0, 1.0], np.float64)
    k = np.outer(k1, k1)
    k = (k / k.sum() * 4).astype(np.float32)
    y = kernel(input=x, kernel=k)
    print("out", y.shape, y.dtype, float(np.abs(y).max()))


# revision 12
# speedup vs baseline: 1.0601x; 1.0601x over previous
"""Depthwise 4x4 separable blur (upfirdn2d pad=(2,1)) on 8 TRN2 NeuronCores.

v3 design — h-pair partitioning, split DMA queues:
  - Pure data parallel over batch: core b gets image b = [C=128, H=256, W=256].
  - SBUF partition p holds H-row pair (2p, 2p+1): every DMA descriptor is a
    2 KB contiguous run, and the whole H extent lives in one tile so the
    H-conv needs no cross-tile seam handling.
  - Input DMAs issue from the Sync HWDGE ring, output DMAs + weight loads
    from the Scalar HWDGE ring, so input prefetch is never queued behind
    output triggers that wait on compute.
  - W-pass on VectorE: two flat scalar_tensor_tensor ops over the whole
    tile (reads that cross a 256-column row boundary produce garbage that
    3 tiny strided DVE fixup ops overwrite — the fixup APs cover every
    boundary). Outputs t1/t2 in fp16.
  - H-pass on TensorE (fp16, 1 cyc/row, fast weight load): for parities
    e, e' the banded matrices B[q][e][ep][p, m] = scale_q * kh[(2p+e) -
    (2m+ep) + 2] give psum[ep] = sum_{q, e} B^T tq[e], PSUM-accumulated.
  - ScalarE interleaves PSUM -> SBUF in (c, e', w) order so output DMA
    descriptors are also 2 KB contiguous.
"""

import os
import sys

import numpy as np

for _p in ("/opt/trn_rl_repo", "/root/.axon_site/_ro/trn_rl_repo"):
    if os.path.isdir(_p) and _p not in sys.path:
        sys.path.append(_p)

import concourse.bacc as bacc
import concourse.mybir as mybir
from concourse import tile
from concourse.bass_utils import run_bass_kernel_spmd

B, C, H, W = 8, 128, 256, 256
N_CORES = 8
CG = 8               # channels per inner tile group
NG = C // CG         # groups
HP = H // 2          # 128 h-pairs = partitions
EW = 2 * W           # flat (e, w) extent per (partition, channel) = 512
FG = CG * EW         # free elements per x/t tile
KS = 4
MM_DT = mybir.dt.float16
OC = 4               # channels per output staging tile / DMA


def _build_bands(kern: np.ndarray):
    """Factor flip(kern) = outer(kh, kw); build the 8 parity band matrices."""
    k = np.flip(kern.astype(np.float64), (0, 1))
    u, s, vt = np.linalg.svd(k)
    assert s[1] < 1e-6 * s[0], "blur kernel must be separable"
    kh = u[:, 0] * np.sqrt(s[0])
    kw = vt[0] * np.sqrt(s[0])
    if kh.sum() < 0:
        kh, kw = -kh, -kw
    assert np.allclose(np.outer(kh, kw), k, atol=1e-12 + 1e-7 * np.abs(k).max())
    assert abs(kw[3]) > 1e-12 and abs(kw[2]) > 1e-12
    r1 = float(kw[0] / kw[3])   # t1 = r1 * x[w-2] + x[w+1]
    r2 = float(kw[1] / kw[2])   # t2 = r2 * x[w-1] + x[w]
    scales = (kw[3], kw[2])     # psum += scale_q * band^T tq

    M = np.zeros((H, H), np.float64)
    for hh in range(H):
        for t in range(KS):
            i = hh + t - 2
            if 0 <= i < H:
                M[i, hh] = kh[t]
    bands = np.zeros((2, 2, 2, HP, HP), np.float64)
    for q in range(2):
        for e in range(2):
            for ep in range(2):
                bands[q, e, ep] = scales[q] * M[e::2, ep::2]
    return bands.reshape(8, HP, HP).astype(np.float32), r1, r2


def _build_nc(r1: float, r2: float):
    nc = bacc.Bacc("TRN2", target_bir_lowering=False, debug=False,
                   num_devices=N_CORES)
    x = nc.dram_tensor("input", [C, H, W], mybir.dt.float32,
                       kind="ExternalInput").ap()
    bands = nc.dram_tensor("bands", [8, HP, HP], mybir.dt.float32,
                           kind="ExternalInput").ap()
    out = nc.dram_tensor("output", [C, H, W], mybir.dt.float32,
                         kind="ExternalOutput").ap()
    mult = mybir.AluOpType.mult
    add = mybir.AluOpType.add

    with tile.TileContext(nc) as tc:
        with (
            tc.tile_pool(name="bands", bufs=1) as bp,
            tc.tile_pool(name="xp", bufs=3) as xpp,
            tc.tile_pool(name="tp", bufs=2) as tp,
            tc.tile_pool(name="osb", bufs=3) as osb,
            tc.tile_pool(name="ps", bufs=8, space="PSUM") as pp,
        ):
            # Band matrices via the Scalar HWDGE ring; cast to fp16 on DVE.
            wm = {}
            for idx in range(8):
                bt = bp.tile([HP, HP], mybir.dt.float32, tag=f"bf{idx}")
                nc.scalar.dma_start(bt[:], bands[idx])
                br = bp.tile([HP, HP], MM_DT, tag=f"br{idx}")
                nc.vector.tensor_copy(br[:], bt[:])
                q, e, ep = idx >> 2, (idx >> 1) & 1, idx & 1
                wm[q, e, ep] = br

            # Taper first/last groups so pipeline fill and drain are short.
            segs = []
            c = 0
            for cg in [4, 4] + [CG] * ((C - 16) // CG) + [4, 4]:
                segs.append((c, cg))
                c += cg
            assert c == C
            for c0, cg in segs:
                fg = cg * EW
                xt = xpp.tile([HP, fg], mybir.dt.float32, tag="x")
                xf = xt[:]
                nc.sync.dma_start(
                    xf.rearrange("p (c f) -> p c f", c=cg),
                    x[c0:c0 + cg].rearrange("c (p e) w -> p c (e w)", e=2),
                )
                t1 = tp.tile([HP, fg], MM_DT, tag="t1")
                t2 = tp.tile([HP, fg], MM_DT, tag="t2")
                t1f, t2f = t1[:], t2[:]
                # Main W-pass: flat ranges over the whole tile; every
                # 256-boundary-corrupted column is rewritten by the fixups.
                nc.vector.scalar_tensor_tensor(
                    t1f[:, 2:fg - 1], xf[:, 0:fg - 3], r1,
                    xf[:, 3:fg], mult, add)
                nc.vector.scalar_tensor_tensor(
                    t2f[:, 1:fg], xf[:, 0:fg - 1], r2,
                    xf[:, 1:fg], mult, add)
                # Fixups (strided 4d views over c and both e rows):
                t1e = t1f.rearrange("p (c pr w) -> p c pr w", c=cg, pr=2)
                t2e = t2f.rearrange("p (c pr w) -> p c pr w", c=cg, pr=2)
                xe = xf.rearrange("p (c pr w) -> p c pr w", c=cg, pr=2)
                # t1[w=0,1] = x[w+1] (left pad kills the r1 term)
                nc.vector.tensor_copy(t1e[:, :, :, 0:2], xe[:, :, :, 1:3])
                # t1[w=255] = r1 * x[w-2] (right pad kills the + term)
                nc.vector.tensor_scalar_mul(
                    t1e[:, :, :, W - 1:W], xe[:, :, :, W - 3:W - 2], r1)
                # t2[w=0] = x[w] (left pad kills the r2 term)
                nc.vector.tensor_copy(t2e[:, :, :, 0:1], xe[:, :, :, 0:1])

                t1c = t1f.rearrange("p (c f) -> p c f", c=cg)
                t2c = t2f.rearrange("p (c f) -> p c f", c=cg)
                for s0 in range(0, cg, OC):
                    oc = min(OC, cg - s0)
                    ot = osb.tile([HP, oc * EW], mybir.dt.float32, tag="o")
                    oc4 = ot[:].rearrange("p (c e w) -> p c e w", c=oc, e=2)
                    for pr in range(oc // 2):
                        cc = s0 + pr * 2
                        for ep in (0, 1):
                            ps = pp.tile([HP, 512], mybir.dt.float32, tag="ps")
                            first = True
                            for q, tt in ((0, t1c), (1, t2c)):
                                for e in (0, 1):
                                    rhs = tt[:, cc:cc + 2,
                                             e * W:(e + 1) * W]
                                    nc.tensor.matmul(
                                        ps[:], wm[q, e, ep][:], rhs,
                                        start=first,
                                        stop=(q == 1 and e == 1))
                                    first = False
                            nc.scalar.copy(
                                oc4[:, pr * 2:pr * 2 + 2, ep, :],
                                ps[:].rearrange("p (c w) -> p c w", c=2),
                            )
                    nc.scalar.dma_start(
                        out[c0 + s0:c0 + s0 + oc]
                        .rearrange("c (p e) w -> p c (e w)", e=2),
                        oc4.rearrange("p c e w -> p c (e w)"),
                    )
    nc.compile()
    return nc


_CACHE = {}


def _get_nc(r1: float, r2: float):
    key = (r1, r2)
    if key not in _CACHE:
        _CACHE[key] = _build_nc(r1, r2)
    return _CACHE[key]


def kernel(**inputs) -> np.ndarray:
    x = np.asarray(inputs["input"], dtype=np.float32)
    kern = np.asarray(inputs["kernel"], dtype=np.float32)
    assert x.shape == (B, C, H, W) and kern.shape == (KS, KS)
    bands, r1, r2 = _build_bands(kern)
    nc = _get_nc(r1, r2)
    in_maps = [
        {"input": np.ascontiguousarray(x[i]), "bands": bands}
        for i in range(N_CORES)
    ]
    res = run_bass_kernel_spmd(nc, in_maps, list(range(N_CORES)))
    global _LAST_RESULTS
    _LAST_RESULTS = res
    return np.stack([res.results[i]["output"] for i in range(N_CORES)])


if __name__ == "__main__":
    rng = np.random.default_rng(0)
    x = rng.standard_normal((B, C, H, W), dtype=np.float32)
    k1 = np.array([1.0, 3.0, 3.0, 1.0], np.float64)
    k = np.outer(k1, k1)
    k = (k / k.sum() * 4).astype(np.float32)
    y = kernel(input=x, kernel=k)
    print("out", y.shape, y.dtype, float(np.abs(y).max()))
